# revision 23
# baseline (speedup 1.0000x reference)
"""Chunkwise SSM layer as a Bass/Tile kernel on 8 Trainium2 NeuronCores.

Math: the reference's inter-chunk correction cancels exactly
(h_next = Th + (h_final - Th) = h_final for ANY mix_weight), so the layer
reduces to a plain diagonal first-order scan:
    G  = sigmoid(x @ gate_W + gate_b)        (B,S,n)
    Bv = x @ B_W                             (B,S,n)
    h_t = G_t * h_{t-1} + Bv_t               (scan over S)
    out = (h @ C_W) * sigmoid(x @ out_W)     (B,S,d)

Sharding: (batch, seq-half) -> 8 cores. Second halves re-derive their
initial state with a W-token warmup scan (gate products decay ~e^-0.05/step;
128 tokens leave a <1e-3 relative dent vs the 2e-2 gate) -- no cross-core
communication. First halves get a zero warmup (exact).

Perf notes (from the f32r + fp8 traces):
  * The kernel is PE-stream bound. f32r already streams 1 col/cycle at
    N>=256, so bf16 does not speed the stream; fp8 DoubleRow would (2x) but
    its 3-bit mantissa fails the 2e-2 gate (measured 4.2e-2), and residual
    compensation passes cost as much as bf16. og stays bf16: ~63us/core.
  * x arrives pre-transposed + plane-tiled from the host, killing the
    baseline's 144 PE transposes (+16us) and their psum->sbuf copies.
  * All DMAs move >=2KB contiguous runs per partition (the flat block
    layout), out_W is split in two halves so the first out-gate chain
    doesn't wait for the full 2MiB, and x/out ride HBM as bf16.
  * y matmul uses a bf16 cast of h: f32r stationaries self-load serially
    (~+180ns per matmul); bf16 weight loads pipeline behind the stream.
  * Sigmoids on Scalar (the only ACT engine; Copy is banished from Scalar
    so the sigmoid table loads exactly once), scan + copies + final muls
    on Vector. GpSimd cannot touch PSUM, so it sits out.
"""

import numpy as np

_B, _S, _D, _N = 4, 4096, 1024, 64
_T = _S // 2  # main tokens per core
_W = 128      # warmup tokens (scan state re-derivation for second halves)
_TB = 512     # tokens per main pipeline block
_BLOCKS = [_W] + [_TB] * (_T // _TB)  # warmup block + 4 main blocks
_KT = _D // 128  # 8 contraction planes of 128

_cache = {}


def _build():
    import concourse.mybir as mybir
    import concourse.tile as tile
    from concourse import bacc

    F32 = mybir.dt.float32
    BF16 = mybir.dt.bfloat16
    Sigmoid = mybir.ActivationFunctionType.Sigmoid
    MULT, ADD = mybir.AluOpType.mult, mybir.AluOpType.add

    nc = bacc.Bacc("TRN2", target_bir_lowering=False, debug=False, num_devices=8)

    # Host-pretiled x^T: per block, [128 partitions, KT planes, TB tokens]
    # flattened per partition so each block load is one contiguous run.
    # The warmup x block rides in `head` together with the gate/B weights so
    # the whole pre-first-matmul working set is ONE dma (descriptor-gen on
    # the ring costs ~0.6us per dma_start, so the startup chain counts them).
    _NW = _KT * 2 * _N  # wgb columns
    head = nc.dram_tensor("head", [128, _NW + _KT * _W], BF16, kind="ExternalInput")
    n_xbf = sum(_KT * tb for tb in _BLOCKS[1:])
    xbf = nc.dram_tensor("xbf", [128, n_xbf], BF16, kind="ExternalInput")
    # out_W reordered [128, 2 halves, KT planes, 512] so each half is one
    # contiguous 8KB-per-partition load
    owr = nc.dram_tensor("owr", [128, 2 * _KT * 512], BF16, kind="ExternalInput")
    cwb = nc.dram_tensor("cwb", [_N, _D], BF16, kind="ExternalInput")
    gbias = nc.dram_tensor("gbias", [_N, 1], F32, kind="ExternalInput")
    out = nc.dram_tensor("out", [_T, _D], BF16, kind="ExternalOutput")

    with tile.TileContext(nc) as tc:
        with (
            tc.tile_pool(name="singles", bufs=1) as singles,
            tc.tile_pool(name="xbfp", bufs=2) as xbf_pool,
            tc.tile_pool(name="gates", bufs=2) as gates_pool,
            tc.tile_pool(name="hpool", bufs=2) as h_pool,
            tc.tile_pool(name="opool", bufs=4) as o_pool,
            tc.tile_pool(name="gb_ps", bufs=2, space="PSUM") as gb_ps,
            tc.tile_pool(name="og_ps", bufs=4, space="PSUM") as og_ps,
            tc.tile_pool(name="y_ps", bufs=2, space="PSUM") as y_ps,
        ):
            # ---- strictly ordered startup loads on the sync HWDGE ring ----
            head_t = singles.tile([128, _NW + _KT * _W], BF16)
            nc.sync.dma_start(out=head_t[:], in_=head.ap())
            wgb_t = head_t[:, :_NW].rearrange("p (o m) -> p o m", o=_KT)

            xbf_off = [0]
            for tb in _BLOCKS[1:]:
                xbf_off.append(xbf_off[-1] + _KT * tb)

            def load_xbf(blk):  # main blocks only (blk >= 1)
                TB = _BLOCKS[blk]
                t = xbf_pool.tile([128, _KT * _TB], BF16, tag="xbf", name="xbf")
                flat = t[:, : _KT * TB]
                nc.sync.dma_start(
                    out=flat, in_=xbf.ap()[:, xbf_off[blk - 1] : xbf_off[blk]]
                )
                return flat.rearrange("p (o t) -> p o t", o=_KT)

            # Block-1 x and out_W half A land interleaved in 0.25MB kk-pair
            # chunks in first-og-chain consumption order (the chunking is only
            # worth the descriptor-gen cost at the critical pipeline head);
            # gate bias, C_W and out_W half B follow.
            pre = {0: head_t[:, _NW:].rearrange("p (o t) -> p o t", o=_KT)}
            x1_t = xbf_pool.tile([128, _KT * _TB], BF16, tag="xbf", name="xbf")
            ow_t = singles.tile([128, 2 * _KT * 512], BF16)
            PAIR = 2 * 512
            for j in range(_KT // 2):
                nc.sync.dma_start(
                    out=x1_t[:, j * PAIR : (j + 1) * PAIR],
                    in_=xbf.ap()[:, j * PAIR : (j + 1) * PAIR],
                )
                nc.sync.dma_start(
                    out=ow_t[:, j * PAIR : (j + 1) * PAIR],
                    in_=owr.ap()[:, j * PAIR : (j + 1) * PAIR],
                )
            pre[1] = x1_t.rearrange("p (o t) -> p o t", o=_KT)
            gb_t = singles.tile([_N, 1], F32)
            nc.sync.dma_start(out=gb_t[:], in_=gbias.ap())
            cw_t = singles.tile([_N, _D], BF16)
            nc.sync.dma_start(out=cw_t[:], in_=cwb.ap())
            nc.sync.dma_start(out=ow_t[:, _KT * 512 :], in_=owr.ap()[:, _KT * 512 :])
            ow_t = ow_t.rearrange("p (c o m) -> p c o m", c=2, o=_KT)

            prev_ht, prev_tb = None, 0
            for blk, TB in enumerate(_BLOCKS):
                xbft = pre.pop(blk, None)
                if xbft is None:
                    xbft = load_xbf(blk)

                # gate/B projections: psum[0:64]=G^T logits, [64:128]=Bv^T
                gbp = gb_ps.tile([128, _TB], F32, tag="gb", name="gbp")[:, :TB]
                for kk in range(_KT):
                    nc.tensor.matmul(
                        gbp[:],
                        wgb_t[:, kk, :],
                        xbft[:, kk, :],
                        start=(kk == 0),
                        stop=(kk == _KT - 1),
                    )
                st = gates_pool.tile([_N, _TB], F32, tag="st", name="st")[:, :TB]
                nc.scalar.activation(
                    out=st[:], in_=gbp[:_N, :], func=Sigmoid, bias=gb_t[:], scale=1.0
                )
                bt = gates_pool.tile([_N, _TB], F32, tag="bt", name="bt")[:, :TB]
                nc.vector.tensor_copy(bt[:], gbp[_N:, :])

                # the scan: h = G*h + Bv along time, chained across blocks
                ht = h_pool.tile([_N, _TB], F32, tag="ht", name="ht")[:, :TB]
                init = 0.0 if prev_ht is None else prev_ht[:, prev_tb - 1 : prev_tb]
                nc.vector.tensor_tensor_scan(
                    ht[:], st[:], bt[:], init, op0=MULT, op1=ADD
                )
                prev_ht, prev_tb = ht, TB

                if blk == 0:
                    continue  # warmup block: only the state matters

                hb = h_pool.tile([_N, _TB], BF16, tag="hb", name="hb")[:, :TB]
                nc.vector.tensor_copy(hb[:], ht[:])

                # out-gate + y + final product, natural [t, d] layout
                NTT = TB // 128
                row0 = sum(_BLOCKS[1:blk])

                def og_chain(ogp, tt, ck):
                    for kk in range(_KT):
                        nc.tensor.matmul(
                            ogp[:],
                            xbft[:, kk, tt * 128 : (tt + 1) * 128],
                            ow_t[:, ck, kk, :],
                            start=(kk == 0),
                            stop=(kk == _KT - 1),
                        )

                def finish(ot, ogp, tt, ck):
                    yp = y_ps.tile([128, 512], F32, tag="y", name="yp")
                    nc.tensor.matmul(
                        yp[:],
                        hb[:, tt * 128 : (tt + 1) * 128],
                        cw_t[:, ck * 512 : (ck + 1) * 512],
                        start=True,
                        stop=True,
                    )
                    cs = slice(ck * 512, (ck + 1) * 512)
                    nc.scalar.activation(
                        out=ot[:, cs], in_=ogp[:], func=Sigmoid, bias=0.0, scale=1.0
                    )
                    nc.vector.tensor_mul(ot[:, cs], ot[:, cs], yp[:])

                if blk == 1:
                    # head of the pipeline: out_W half B is still in flight, so
                    # run ALL ck=0 chains first, then ck=1 + finish per tile
                    ots = [
                        o_pool.tile([128, _D], BF16, tag="ot", name="ot")
                        for _ in range(NTT)
                    ]
                    og0 = []
                    for tt in range(NTT):
                        ogp = og_ps.tile([128, 512], F32, tag="og", name="ogp")
                        og_chain(ogp, tt, 0)
                        og0.append(ogp)
                    for tt in range(NTT):
                        ogp1 = og_ps.tile([128, 512], F32, tag="og", name="ogp")
                        og_chain(ogp1, tt, 1)
                        finish(ots[tt], og0[tt], tt, 0)
                        finish(ots[tt], ogp1, tt, 1)
                        row = row0 + tt * 128
                        nc.scalar.dma_start(
                            out=out.ap()[row : row + 128, :], in_=ots[tt][:]
                        )
                else:
                    for tt in range(NTT):
                        ot = o_pool.tile([128, _D], BF16, tag="ot", name="ot")
                        ogps = [
                            og_ps.tile([128, 512], F32, tag="og", name="ogp")
                            for _ in range(2)
                        ]
                        for kk in range(_KT):
                            lhs = xbft[:, kk, tt * 128 : (tt + 1) * 128]
                            for ck in range(2):
                                nc.tensor.matmul(
                                    ogps[ck][:],
                                    lhs,
                                    ow_t[:, ck, kk, :],
                                    start=(kk == 0),
                                    stop=(kk == _KT - 1),
                                )
                        row = row0 + tt * 128
                        if blk == len(_BLOCKS) - 1 and tt == NTT - 1:
                            # tail: store each half as soon as its mul lands
                            for ck in range(2):
                                finish(ot, ogps[ck], tt, ck)
                                cs = slice(ck * 512, (ck + 1) * 512)
                                nc.scalar.dma_start(
                                    out=out.ap()[row : row + 128, cs],
                                    in_=ot[:, cs],
                                )
                        else:
                            for ck in range(2):
                                finish(ot, ogps[ck], tt, ck)
                            nc.scalar.dma_start(
                                out=out.ap()[row : row + 128, :], in_=ot[:]
                            )
    nc.compile()
    return nc


def _pretile(w):  # [d, m] -> [128, (d//128) * m], contraction planes on partitions
    d, m = w.shape
    return np.ascontiguousarray(
        w.reshape(d // 128, 128, m).transpose(1, 0, 2).reshape(128, -1)
    )


def kernel(x, gate_W, gate_b, B_W, C_W, out_W, mix_weight, chunk_size):
    import ml_dtypes
    from concourse.bass_utils import run_bass_kernel_spmd

    BF16 = ml_dtypes.bfloat16

    x = np.ascontiguousarray(np.asarray(x), dtype=np.float32)
    assert x.shape == (_B, _S, _D), x.shape

    nc = _cache.get("nc")
    if nc is None:
        nc = _cache["nc"] = _build()

    wgb = _pretile(
        np.concatenate(
            [np.asarray(gate_W, np.float32), np.asarray(B_W, np.float32)], axis=1
        )
    ).astype(BF16)  # [128, KT*2N], packed into head with the warmup x block
    # [128, ck, kk, 512]: per-partition-contiguous halves of pretiled out_W
    ow = _pretile(np.asarray(out_W, np.float32)).reshape(128, _KT, 2, 512)
    owr = np.ascontiguousarray(ow.transpose(0, 2, 1, 3).reshape(128, -1)).astype(BF16)
    cwb = np.ascontiguousarray(np.asarray(C_W, np.float32)).astype(BF16)
    gbias = np.ascontiguousarray(np.asarray(gate_b, np.float32).reshape(_N, 1))

    zeros_warm = np.zeros((_W, _D), np.float32)
    in_maps = []
    for b in range(_B):
        for half in range(2):
            main = x[b, half * _T : (half + 1) * _T]
            warm = zeros_warm if half == 0 else x[b, _T - _W : _T]
            xs = np.concatenate([warm, main], axis=0)  # [W+T, D]
            planes = xs.T.reshape(_KT, 128, _W + _T)
            chunks, off = [], 0
            for tb in _BLOCKS:
                blkv = planes[:, :, off : off + tb]  # [KT, 128, tb]
                chunks.append(blkv.transpose(1, 0, 2).reshape(128, -1))
                off += tb
            headm = np.ascontiguousarray(
                np.concatenate([wgb.astype(np.float32), chunks[0]], axis=1)
            ).astype(BF16)
            xbf = np.ascontiguousarray(np.concatenate(chunks[1:], axis=1)).astype(
                BF16
            )
            in_maps.append(
                dict(head=headm, xbf=xbf, owr=owr, cwb=cwb, gbias=gbias)
            )

    res = run_bass_kernel_spmd(nc, in_maps, core_ids=list(range(8)))
    _cache["last_result"] = res

    out = np.empty((_B, _S, _D), np.float32)
    for i in range(8):
        b, half = divmod(i, 2)
        out[b, half * _T : (half + 1) * _T] = res.results[i]["out"].astype(np.float32)
    return out


# revision 24
# speedup vs baseline: 1.1465x; 1.1465x over previous
"""Chunkwise SSM layer as a Bass/Tile kernel on 8 Trainium2 NeuronCores.

Math: the reference's inter-chunk correction cancels exactly
(h_next = Th + (h_final - Th) = h_final for ANY mix_weight), so the layer
reduces to a plain diagonal first-order scan:
    G  = sigmoid(x @ gate_W + gate_b)        (B,S,n)
    Bv = x @ B_W                             (B,S,n)
    h_t = G_t * h_{t-1} + Bv_t               (scan over S)
    out = (h @ C_W) * sigmoid(x @ out_W)     (B,S,d)

Sharding: (batch, seq-half) -> 8 cores. Second halves re-derive their
initial state with a W-token warmup scan (gate products decay ~e^-0.05/step;
128 tokens leave a <1e-3 relative dent vs the 2e-2 gate) -- no cross-core
communication. First halves get a zero warmup (exact).

Perf notes (from the f32r + fp8 traces):
  * The kernel is PE-stream bound. f32r already streams 1 col/cycle at
    N>=256, so bf16 does not speed the stream; fp8 DoubleRow would (2x) but
    its 3-bit mantissa fails the 2e-2 gate (measured 4.2e-2), and residual
    compensation passes cost as much as bf16. og stays bf16: ~63us/core.
  * x arrives pre-transposed + plane-tiled from the host, killing the
    baseline's 144 PE transposes (+16us) and their psum->sbuf copies.
  * All DMAs move >=2KB contiguous runs per partition (the flat block
    layout), out_W is split in two halves so the first out-gate chain
    doesn't wait for the full 2MiB, and x/out ride HBM as bf16.
  * y matmul uses a bf16 cast of h: f32r stationaries self-load serially
    (~+180ns per matmul); bf16 weight loads pipeline behind the stream.
  * Sigmoids on Scalar (the only ACT engine; Copy is banished from Scalar
    so the sigmoid table loads exactly once), scan + copies + final muls
    on Vector. GpSimd cannot touch PSUM, so it sits out.
"""

import numpy as np

_B, _S, _D, _N = 4, 4096, 1024, 64
_T = _S // 2  # main tokens per core
_W = 128      # warmup tokens (scan state re-derivation for second halves)
_TB = 512     # tokens per main pipeline block
_BLOCKS = [_W] + [_TB] * (_T // _TB)  # warmup block + 4 main blocks
_KT = _D // 128  # 8 contraction planes of 128

_cache = {}


def _build():
    import concourse.mybir as mybir
    import concourse.tile as tile
    from concourse import bacc

    F32 = mybir.dt.float32
    BF16 = mybir.dt.bfloat16
    Sigmoid = mybir.ActivationFunctionType.Sigmoid
    MULT, ADD = mybir.AluOpType.mult, mybir.AluOpType.add

    nc = bacc.Bacc("TRN2", target_bir_lowering=False, debug=False, num_devices=8)

    # Host-pretiled x^T: per block, [128 partitions, KT planes, TB tokens]
    # flattened per partition so each block load is one contiguous run.
    # The warmup x block rides in `head` together with the gate/B weights so
    # the whole pre-first-matmul working set is ONE dma (descriptor-gen on
    # the ring costs ~0.6us per dma_start, so the startup chain counts them).
    _NW = _KT * 2 * _N  # wgb columns
    head = nc.dram_tensor("head", [128, _NW + _KT * _W], BF16, kind="ExternalInput")
    n_xbf = sum(_KT * tb for tb in _BLOCKS[1:])
    xbf = nc.dram_tensor("xbf", [128, n_xbf], BF16, kind="ExternalInput")
    # out_W reordered [128, 2 halves, KT planes, 512] so each half is one
    # contiguous 8KB-per-partition load
    owr = nc.dram_tensor("owr", [128, 2 * _KT * 512], BF16, kind="ExternalInput")
    cwb = nc.dram_tensor("cwb", [_N, _D], BF16, kind="ExternalInput")
    gbias = nc.dram_tensor("gbias", [_N, 1], F32, kind="ExternalInput")
    out = nc.dram_tensor("out", [_T, _D], BF16, kind="ExternalOutput")

    with tile.TileContext(nc) as tc:
        with (
            tc.tile_pool(name="singles", bufs=1) as singles,
            tc.tile_pool(name="xbfp", bufs=2) as xbf_pool,
            tc.tile_pool(name="gates", bufs=2) as gates_pool,
            tc.tile_pool(name="hpool", bufs=2) as h_pool,
            tc.tile_pool(name="opool", bufs=4) as o_pool,
            tc.tile_pool(name="gb_ps", bufs=2, space="PSUM") as gb_ps,
            tc.tile_pool(name="og_ps", bufs=4, space="PSUM") as og_ps,
            tc.tile_pool(name="y_ps", bufs=2, space="PSUM") as y_ps,
        ):
            # ---- strictly ordered startup loads on the sync HWDGE ring ----
            head_t = singles.tile([128, _NW + _KT * _W], BF16)
            nc.sync.dma_start(out=head_t[:], in_=head.ap())
            wgb_t = head_t[:, :_NW].rearrange("p (o m) -> p o m", o=_KT)

            xbf_off = [0]
            for tb in _BLOCKS[1:]:
                xbf_off.append(xbf_off[-1] + _KT * tb)

            def load_xbf(blk):  # main blocks only (blk >= 1)
                TB = _BLOCKS[blk]
                t = xbf_pool.tile([128, _KT * _TB], BF16, tag="xbf", name="xbf")
                flat = t[:, : _KT * TB]
                nc.sync.dma_start(
                    out=flat, in_=xbf.ap()[:, xbf_off[blk - 1] : xbf_off[blk]]
                )
                return flat.rearrange("p (o t) -> p o t", o=_KT)

            # Block-1 x and out_W half A land interleaved in 0.25MB kk-pair
            # chunks in first-og-chain consumption order (the chunking is only
            # worth the descriptor-gen cost at the critical pipeline head);
            # gate bias, C_W and out_W half B follow.
            pre = {0: head_t[:, _NW:].rearrange("p (o t) -> p o t", o=_KT)}
            x1_t = xbf_pool.tile([128, _KT * _TB], BF16, tag="xbf", name="xbf")
            ow_t = singles.tile([128, 2 * _KT * 512], BF16)
            PAIR = 2 * 512
            for j in range(_KT // 2):
                nc.sync.dma_start(
                    out=x1_t[:, j * PAIR : (j + 1) * PAIR],
                    in_=xbf.ap()[:, j * PAIR : (j + 1) * PAIR],
                )
                nc.scalar.dma_start(
                    out=ow_t[:, j * PAIR : (j + 1) * PAIR],
                    in_=owr.ap()[:, j * PAIR : (j + 1) * PAIR],
                )
            pre[1] = x1_t.rearrange("p (o t) -> p o t", o=_KT)
            gb_t = singles.tile([_N, 1], F32)
            nc.sync.dma_start(out=gb_t[:], in_=gbias.ap())
            cw_t = singles.tile([_N, _D], BF16)
            nc.scalar.dma_start(out=cw_t[:], in_=cwb.ap())
            nc.scalar.dma_start(
                out=ow_t[:, _KT * 512 :], in_=owr.ap()[:, _KT * 512 :]
            )
            ow_t = ow_t.rearrange("p (c o m) -> p c o m", c=2, o=_KT)

            prev_ht, prev_tb = None, 0
            for blk, TB in enumerate(_BLOCKS):
                xbft = pre.pop(blk, None)
                if xbft is None:
                    xbft = load_xbf(blk)

                # gate/B projections: psum[0:64]=G^T logits, [64:128]=Bv^T
                gbp = gb_ps.tile([128, _TB], F32, tag="gb", name="gbp")[:, :TB]
                for kk in range(_KT):
                    nc.tensor.matmul(
                        gbp[:],
                        wgb_t[:, kk, :],
                        xbft[:, kk, :],
                        start=(kk == 0),
                        stop=(kk == _KT - 1),
                    )
                st = gates_pool.tile([_N, _TB], F32, tag="st", name="st")[:, :TB]
                nc.scalar.activation(
                    out=st[:], in_=gbp[:_N, :], func=Sigmoid, bias=gb_t[:], scale=1.0
                )
                bt = gates_pool.tile([_N, _TB], F32, tag="bt", name="bt")[:, :TB]
                nc.vector.tensor_copy(bt[:], gbp[_N:, :])

                # the scan: h = G*h + Bv along time, chained across blocks
                ht = h_pool.tile([_N, _TB], F32, tag="ht", name="ht")[:, :TB]
                init = 0.0 if prev_ht is None else prev_ht[:, prev_tb - 1 : prev_tb]
                nc.vector.tensor_tensor_scan(
                    ht[:], st[:], bt[:], init, op0=MULT, op1=ADD
                )
                prev_ht, prev_tb = ht, TB

                if blk == 0:
                    continue  # warmup block: only the state matters

                hb = h_pool.tile([_N, _TB], BF16, tag="hb", name="hb")[:, :TB]
                nc.vector.tensor_copy(hb[:], ht[:])

                # out-gate + y + final product, natural [t, d] layout
                NTT = TB // 128
                row0 = sum(_BLOCKS[1:blk])

                def og_chain(ogp, tt, ck):
                    for kk in range(_KT):
                        nc.tensor.matmul(
                            ogp[:],
                            xbft[:, kk, tt * 128 : (tt + 1) * 128],
                            ow_t[:, ck, kk, :],
                            start=(kk == 0),
                            stop=(kk == _KT - 1),
                        )

                def finish(ot, ogp, tt, ck):
                    yp = y_ps.tile([128, 512], F32, tag="y", name="yp")
                    nc.tensor.matmul(
                        yp[:],
                        hb[:, tt * 128 : (tt + 1) * 128],
                        cw_t[:, ck * 512 : (ck + 1) * 512],
                        start=True,
                        stop=True,
                    )
                    cs = slice(ck * 512, (ck + 1) * 512)
                    nc.scalar.activation(
                        out=ot[:, cs], in_=ogp[:], func=Sigmoid, bias=0.0, scale=1.0
                    )
                    nc.vector.tensor_mul(ot[:, cs], ot[:, cs], yp[:])

                if blk == 1:
                    # head of the pipeline: out_W half B is still in flight, so
                    # run ALL ck=0 chains first, then ck=1 + finish per tile
                    ots = [
                        o_pool.tile([128, _D], BF16, tag="ot", name="ot")
                        for _ in range(NTT)
                    ]
                    og0 = []
                    for tt in range(NTT):
                        ogp = og_ps.tile([128, 512], F32, tag="og", name="ogp")
                        og_chain(ogp, tt, 0)
                        og0.append(ogp)
                    for tt in range(NTT):
                        ogp1 = og_ps.tile([128, 512], F32, tag="og", name="ogp")
                        og_chain(ogp1, tt, 1)
                        finish(ots[tt], og0[tt], tt, 0)
                        finish(ots[tt], ogp1, tt, 1)
                        row = row0 + tt * 128
                        nc.scalar.dma_start(
                            out=out.ap()[row : row + 128, :], in_=ots[tt][:]
                        )
                else:
                    for tt in range(NTT):
                        ot = o_pool.tile([128, _D], BF16, tag="ot", name="ot")
                        ogps = [
                            og_ps.tile([128, 512], F32, tag="og", name="ogp")
                            for _ in range(2)
                        ]
                        for kk in range(_KT):
                            lhs = xbft[:, kk, tt * 128 : (tt + 1) * 128]
                            for ck in range(2):
                                nc.tensor.matmul(
                                    ogps[ck][:],
                                    lhs,
                                    ow_t[:, ck, kk, :],
                                    start=(kk == 0),
                                    stop=(kk == _KT - 1),
                                )
                        row = row0 + tt * 128
                        if blk == len(_BLOCKS) - 1 and tt == NTT - 1:
                            # tail: store each half as soon as its mul lands
                            for ck in range(2):
                                finish(ot, ogps[ck], tt, ck)
                                cs = slice(ck * 512, (ck + 1) * 512)
                                nc.scalar.dma_start(
                                    out=out.ap()[row : row + 128, cs],
                                    in_=ot[:, cs],
                                )
                        else:
                            for ck in range(2):
                                finish(ot, ogps[ck], tt, ck)
                            nc.scalar.dma_start(
                                out=out.ap()[row : row + 128, :], in_=ot[:]
                            )
    nc.compile()
    return nc


def _pretile(w):  # [d, m] -> [128, (d//128) * m], contraction planes on partitions
    d, m = w.shape
    return np.ascontiguousarray(
        w.reshape(d // 128, 128, m).transpose(1, 0, 2).reshape(128, -1)
    )


def kernel(x, gate_W, gate_b, B_W, C_W, out_W, mix_weight, chunk_size):
    import ml_dtypes
    from concourse.bass_utils import run_bass_kernel_spmd

    BF16 = ml_dtypes.bfloat16

    x = np.ascontiguousarray(np.asarray(x), dtype=np.float32)
    assert x.shape == (_B, _S, _D), x.shape

    nc = _cache.get("nc")
    if nc is None:
        nc = _cache["nc"] = _build()

    wgb = _pretile(
        np.concatenate(
            [np.asarray(gate_W, np.float32), np.asarray(B_W, np.float32)], axis=1
        )
    ).astype(BF16)  # [128, KT*2N], packed into head with the warmup x block
    # [128, ck, kk, 512]: per-partition-contiguous halves of pretiled out_W
    ow = _pretile(np.asarray(out_W, np.float32)).reshape(128, _KT, 2, 512)
    owr = np.ascontiguousarray(ow.transpose(0, 2, 1, 3).reshape(128, -1)).astype(BF16)
    cwb = np.ascontiguousarray(np.asarray(C_W, np.float32)).astype(BF16)
    gbias = np.ascontiguousarray(np.asarray(gate_b, np.float32).reshape(_N, 1))

    zeros_warm = np.zeros((_W, _D), np.float32)
    in_maps = []
    for b in range(_B):
        for half in range(2):
            main = x[b, half * _T : (half + 1) * _T]
            warm = zeros_warm if half == 0 else x[b, _T - _W : _T]
            xs = np.concatenate([warm, main], axis=0)  # [W+T, D]
            planes = xs.T.reshape(_KT, 128, _W + _T)
            chunks, off = [], 0
            for tb in _BLOCKS:
                blkv = planes[:, :, off : off + tb]  # [KT, 128, tb]
                chunks.append(blkv.transpose(1, 0, 2).reshape(128, -1))
                off += tb
            headm = np.ascontiguousarray(
                np.concatenate([wgb.astype(np.float32), chunks[0]], axis=1)
            ).astype(BF16)
            xbf = np.ascontiguousarray(np.concatenate(chunks[1:], axis=1)).astype(
                BF16
            )
            in_maps.append(
                dict(head=headm, xbf=xbf, owr=owr, cwb=cwb, gbias=gbias)
            )

    res = run_bass_kernel_spmd(nc, in_maps, core_ids=list(range(8)))
    _cache["last_result"] = res

    out = np.empty((_B, _S, _D), np.float32)
    for i in range(8):
        b, half = divmod(i, 2)
        out[b, half * _T : (half + 1) * _T] = res.results[i]["out"].astype(np.float32)
    return out


# revision 25
# speedup vs baseline: 1.1842x; 1.0329x over previous
"""Chunkwise SSM layer as a Bass/Tile kernel on 8 Trainium2 NeuronCores.

Math: the reference's inter-chunk correction cancels exactly
(h_next = Th + (h_final - Th) = h_final for ANY mix_weight), so the layer
reduces to a plain diagonal first-order scan:
    G  = sigmoid(x @ gate_W + gate_b)        (B,S,n)
    Bv = x @ B_W                             (B,S,n)
    h_t = G_t * h_{t-1} + Bv_t               (scan over S)
    out = (h @ C_W) * sigmoid(x @ out_W)     (B,S,d)

Sharding: (batch, seq-half) -> 8 cores. Second halves re-derive their
initial state with a W-token warmup scan (gate products decay ~e^-0.05/step;
128 tokens leave a <1e-3 relative dent vs the 2e-2 gate) -- no cross-core
communication. First halves get a zero warmup (exact).

Perf notes (from the f32r + fp8 traces):
  * The kernel is PE-stream bound. f32r already streams 1 col/cycle at
    N>=256, so bf16 does not speed the stream; fp8 DoubleRow would (2x) but
    its 3-bit mantissa fails the 2e-2 gate (measured 4.2e-2), and residual
    compensation passes cost as much as bf16. og stays bf16: ~63us/core.
  * x arrives pre-transposed + plane-tiled from the host, killing the
    baseline's 144 PE transposes (+16us) and their psum->sbuf copies.
  * All DMAs move >=2KB contiguous runs per partition (the flat block
    layout), out_W is split in two halves so the first out-gate chain
    doesn't wait for the full 2MiB, and x/out ride HBM as bf16.
  * y matmul uses a bf16 cast of h: f32r stationaries self-load serially
    (~+180ns per matmul); bf16 weight loads pipeline behind the stream.
  * Sigmoids on Scalar (the only ACT engine; Copy is banished from Scalar
    so the sigmoid table loads exactly once), scan + copies + final muls
    on Vector. GpSimd cannot touch PSUM, so it sits out.
"""

import numpy as np

_B, _S, _D, _N = 4, 4096, 1024, 64
_T = _S // 2  # main tokens per core
_W = 128      # warmup tokens (scan state re-derivation for second halves)
_TB = 512     # tokens per main pipeline block
_BLOCKS = [_W] + [_TB] * (_T // _TB)  # warmup block + 4 main blocks
_KT = _D // 128  # 8 contraction planes of 128

_cache = {}


def _build():
    import concourse.mybir as mybir
    import concourse.tile as tile
    from concourse import bacc

    F32 = mybir.dt.float32
    BF16 = mybir.dt.bfloat16
    Sigmoid = mybir.ActivationFunctionType.Sigmoid
    MULT, ADD = mybir.AluOpType.mult, mybir.AluOpType.add

    nc = bacc.Bacc("TRN2", target_bir_lowering=False, debug=False, num_devices=8)

    # Host-pretiled x^T: per block, [128 partitions, KT planes, TB tokens]
    # flattened per partition so each block load is one contiguous run.
    # The warmup x block rides in `head` together with the gate/B weights so
    # the whole pre-first-matmul working set is ONE dma (descriptor-gen on
    # the ring costs ~0.6us per dma_start, so the startup chain counts them).
    _NW = _KT * 2 * _N  # wgb columns
    head = nc.dram_tensor("head", [128, _NW + _KT * _W], BF16, kind="ExternalInput")
    n_xbf = sum(_KT * tb for tb in _BLOCKS[1:])
    xbf = nc.dram_tensor("xbf", [128, n_xbf], BF16, kind="ExternalInput")
    # out_W reordered [128, 2 halves, KT planes, 512] so each half is one
    # contiguous 8KB-per-partition load
    owr = nc.dram_tensor("owr", [128, 2 * _KT * 512], BF16, kind="ExternalInput")
    cwb = nc.dram_tensor("cwb", [_N, _D], BF16, kind="ExternalInput")
    gbias = nc.dram_tensor("gbias", [_N, 1], F32, kind="ExternalInput")
    out = nc.dram_tensor("out", [_T, _D], BF16, kind="ExternalOutput")

    with tile.TileContext(nc) as tc:
        with (
            tc.tile_pool(name="singles", bufs=1) as singles,
            tc.tile_pool(name="xbfp", bufs=2) as xbf_pool,
            tc.tile_pool(name="gates", bufs=2) as gates_pool,
            tc.tile_pool(name="hpool", bufs=2) as h_pool,
            tc.tile_pool(name="opool", bufs=4) as o_pool,
            tc.tile_pool(name="gb_ps", bufs=2, space="PSUM") as gb_ps,
            tc.tile_pool(name="og_ps", bufs=4, space="PSUM") as og_ps,
            tc.tile_pool(name="y_ps", bufs=2, space="PSUM") as y_ps,
        ):
            # ---- strictly ordered startup loads on the sync HWDGE ring ----
            head_t = singles.tile([128, _NW + _KT * _W], BF16)
            nc.sync.dma_start(out=head_t[:], in_=head.ap())
            wgb_t = head_t[:, :_NW].rearrange("p (o m) -> p o m", o=_KT)

            xbf_off = [0]
            for tb in _BLOCKS[1:]:
                xbf_off.append(xbf_off[-1] + _KT * tb)

            def load_xbf(blk):  # main blocks only (blk >= 1)
                TB = _BLOCKS[blk]
                t = xbf_pool.tile([128, _KT * _TB], BF16, tag="xbf", name="xbf")
                flat = t[:, : _KT * TB]
                nc.sync.dma_start(
                    out=flat, in_=xbf.ap()[:, xbf_off[blk - 1] : xbf_off[blk]]
                )
                return flat.rearrange("p (o t) -> p o t", o=_KT)

            # Block-1 x and out_W half A land interleaved in 0.25MB kk-pair
            # chunks in first-og-chain consumption order (the chunking is only
            # worth the descriptor-gen cost at the critical pipeline head);
            # gate bias, C_W and out_W half B follow.
            pre = {0: head_t[:, _NW:].rearrange("p (o t) -> p o t", o=_KT)}
            x1_t = xbf_pool.tile([128, _KT * _TB], BF16, tag="xbf", name="xbf")
            ow_t = singles.tile([128, 2 * _KT * 512], BF16)
            PAIR = 2 * 512
            for j in range(_KT // 2):
                nc.sync.dma_start(
                    out=x1_t[:, j * PAIR : (j + 1) * PAIR],
                    in_=xbf.ap()[:, j * PAIR : (j + 1) * PAIR],
                )
                nc.sync.dma_start(
                    out=ow_t[:, j * PAIR : (j + 1) * PAIR],
                    in_=owr.ap()[:, j * PAIR : (j + 1) * PAIR],
                )
            pre[1] = x1_t.rearrange("p (o t) -> p o t", o=_KT)
            gb_t = singles.tile([_N, 1], F32)
            nc.sync.dma_start(out=gb_t[:], in_=gbias.ap())
            cw_t = singles.tile([_N, _D], BF16)
            nc.sync.dma_start(out=cw_t[:], in_=cwb.ap())
            nc.sync.dma_start(out=ow_t[:, _KT * 512 :], in_=owr.ap()[:, _KT * 512 :])
            ow_t = ow_t.rearrange("p (c o m) -> p c o m", c=2, o=_KT)

            prev_ht, prev_tb = None, 0
            for blk, TB in enumerate(_BLOCKS):
                xbft = pre.pop(blk, None)
                if xbft is None:
                    xbft = load_xbf(blk)

                # gate/B projections: psum[0:64]=G^T logits, [64:128]=Bv^T
                gbp = gb_ps.tile([128, _TB], F32, tag="gb", name="gbp")[:, :TB]
                for kk in range(_KT):
                    nc.tensor.matmul(
                        gbp[:],
                        wgb_t[:, kk, :],
                        xbft[:, kk, :],
                        start=(kk == 0),
                        stop=(kk == _KT - 1),
                    )
                st = gates_pool.tile([_N, _TB], F32, tag="st", name="st")[:, :TB]
                nc.scalar.activation(
                    out=st[:], in_=gbp[:_N, :], func=Sigmoid, bias=gb_t[:], scale=1.0
                )
                bt = gates_pool.tile([_N, _TB], F32, tag="bt", name="bt")[:, :TB]
                nc.vector.tensor_copy(bt[:], gbp[_N:, :])

                # the scan: h = G*h + Bv along time, chained across blocks
                ht = h_pool.tile([_N, _TB], F32, tag="ht", name="ht")[:, :TB]
                init = 0.0 if prev_ht is None else prev_ht[:, prev_tb - 1 : prev_tb]
                nc.vector.tensor_tensor_scan(
                    ht[:], st[:], bt[:], init, op0=MULT, op1=ADD
                )
                prev_ht, prev_tb = ht, TB

                if blk == 0:
                    continue  # warmup block: only the state matters

                hb = h_pool.tile([_N, _TB], BF16, tag="hb", name="hb")[:, :TB]
                nc.vector.tensor_copy(hb[:], ht[:])

                # out-gate + y + final product, natural [t, d] layout
                NTT = TB // 128
                row0 = sum(_BLOCKS[1:blk])

                def og_chain(ogp, tt, ck):
                    for kk in range(_KT):
                        nc.tensor.matmul(
                            ogp[:],
                            xbft[:, kk, tt * 128 : (tt + 1) * 128],
                            ow_t[:, ck, kk, :],
                            start=(kk == 0),
                            stop=(kk == _KT - 1),
                        )

                def finish(ot, ogp, tt, ck):
                    yp = y_ps.tile([128, 512], F32, tag="y", name="yp")
                    nc.tensor.matmul(
                        yp[:],
                        hb[:, tt * 128 : (tt + 1) * 128],
                        cw_t[:, ck * 512 : (ck + 1) * 512],
                        start=True,
                        stop=True,
                    )
                    cs = slice(ck * 512, (ck + 1) * 512)
                    nc.scalar.activation(
                        out=ot[:, cs], in_=ogp[:], func=Sigmoid, bias=0.0, scale=1.0
                    )
                    nc.vector.tensor_mul(ot[:, cs], ot[:, cs], yp[:])

                if blk == 1:
                    # head of the pipeline: out_W half B is still in flight, so
                    # run ALL ck=0 chains first, then ck=1 + finish per tile
                    ots = [
                        o_pool.tile([128, _D], BF16, tag="ot", name="ot")
                        for _ in range(NTT)
                    ]
                    og0 = []
                    for tt in range(NTT):
                        ogp = og_ps.tile([128, 512], F32, tag="og", name="ogp")
                        og_chain(ogp, tt, 0)
                        og0.append(ogp)
                    for tt in range(NTT):
                        ogp1 = og_ps.tile([128, 512], F32, tag="og", name="ogp")
                        og_chain(ogp1, tt, 1)
                        finish(ots[tt], og0[tt], tt, 0)
                        finish(ots[tt], ogp1, tt, 1)
                        row = row0 + tt * 128
                        nc.scalar.dma_start(
                            out=out.ap()[row : row + 128, :], in_=ots[tt][:]
                        )
                else:
                    for tt in range(NTT):
                        ot = o_pool.tile([128, _D], BF16, tag="ot", name="ot")
                        ogps = [
                            og_ps.tile([128, 512], F32, tag="og", name="ogp")
                            for _ in range(2)
                        ]
                        for kk in range(_KT):
                            lhs = xbft[:, kk, tt * 128 : (tt + 1) * 128]
                            for ck in range(2):
                                nc.tensor.matmul(
                                    ogps[ck][:],
                                    lhs,
                                    ow_t[:, ck, kk, :],
                                    start=(kk == 0),
                                    stop=(kk == _KT - 1),
                                )
                        row = row0 + tt * 128
                        if blk == len(_BLOCKS) - 1 and tt == NTT - 1:
                            # tail: store each half as soon as its mul lands
                            for ck in range(2):
                                finish(ot, ogps[ck], tt, ck)
                                cs = slice(ck * 512, (ck + 1) * 512)
                                nc.scalar.dma_start(
                                    out=out.ap()[row : row + 128, cs],
                                    in_=ot[:, cs],
                                )
                        else:
                            for ck in range(2):
                                finish(ot, ogps[ck], tt, ck)
                            nc.scalar.dma_start(
                                out=out.ap()[row : row + 128, :], in_=ot[:]
                            )
    nc.compile()
    return nc


def _pretile(w):  # [d, m] -> [128, (d//128) * m], contraction planes on partitions
    d, m = w.shape
    return np.ascontiguousarray(
        w.reshape(d // 128, 128, m).transpose(1, 0, 2).reshape(128, -1)
    )


def kernel(x, gate_W, gate_b, B_W, C_W, out_W, mix_weight, chunk_size):
    import ml_dtypes
    from concourse.bass_utils import run_bass_kernel_spmd

    BF16 = ml_dtypes.bfloat16

    x = np.ascontiguousarray(np.asarray(x), dtype=np.float32)
    assert x.shape == (_B, _S, _D), x.shape

    nc = _cache.get("nc")
    if nc is None:
        nc = _cache["nc"] = _build()

    wgb = _pretile(
        np.concatenate(
            [np.asarray(gate_W, np.float32), np.asarray(B_W, np.float32)], axis=1
        )
    ).astype(BF16)  # [128, KT*2N], packed into head with the warmup x block
    # [128, ck, kk, 512]: per-partition-contiguous halves of pretiled out_W
    ow = _pretile(np.asarray(out_W, np.float32)).reshape(128, _KT, 2, 512)
    owr = np.ascontiguousarray(ow.transpose(0, 2, 1, 3).reshape(128, -1)).astype(BF16)
    cwb = np.ascontiguousarray(np.asarray(C_W, np.float32)).astype(BF16)
    gbias = np.ascontiguousarray(np.asarray(gate_b, np.float32).reshape(_N, 1))

    zeros_warm = np.zeros((_W, _D), np.float32)
    in_maps = []
    for b in range(_B):
        for half in range(2):
            main = x[b, half * _T : (half + 1) * _T]
            warm = zeros_warm if half == 0 else x[b, _T - _W : _T]
            xs = np.concatenate([warm, main], axis=0)  # [W+T, D]
            planes = xs.T.reshape(_KT, 128, _W + _T)
            chunks, off = [], 0
            for tb in _BLOCKS:
                blkv = planes[:, :, off : off + tb]  # [KT, 128, tb]
                chunks.append(blkv.transpose(1, 0, 2).reshape(128, -1))
                off += tb
            headm = np.ascontiguousarray(
                np.concatenate([wgb.astype(np.float32), chunks[0]], axis=1)
            ).astype(BF16)
            xbf = np.ascontiguousarray(np.concatenate(chunks[1:], axis=1)).astype(
                BF16
            )
            in_maps.append(
                dict(head=headm, xbf=xbf, owr=owr, cwb=cwb, gbias=gbias)
            )

    res = run_bass_kernel_spmd(nc, in_maps, core_ids=list(range(8)))
    _cache["last_result"] = res

    out = np.empty((_B, _S, _D), np.float32)
    for i in range(8):
        b, half = divmod(i, 2)
        out[b, half * _T : (half + 1) * _T] = res.results[i]["out"].astype(np.float32)
    return out
